# revision 44
# baseline (speedup 1.0000x reference)
"""Trainium2 Bass kernel for nn_LossFunction_46720654246163.

Contrastive (SimCLR-style) loss over N=8192 rows:
  feat = concat(view0, view1) rows, fn = feat / ||feat||
  S = fn @ fn.T  [N,N];  logits = w*S + b;  masked softmax per row
  loss = mean_i [ ln(sum_{j!=i} exp(w*S_ij)) - w*S_ipos ]   (shift-invariant)
  prec1 = 100 * mean_i [ argmax_{j!=i} S_ij == pos(i) ],  pos(i)=(i+N/2)%N

Row-parallel across 8 NeuronCores; the host rotates row order per core so all
cores run the IDENTICAL program (own rows at columns [0,1024), positives at a
fixed +4096 offset). Scalar means are order-invariant -> no un-rotation.

Per core (fp16 nat / bf16 matmuls, UNSHIFTED exp), q-outer m-inner schedule:
 - chunk DMAs spread over the 3 DMA-capable engine queues (parallel HWDGE),
   early chunks split 3-way; consts issued after; PE warmed by dummy matmuls
   during the DMA wait so the HAM activity monitor upclocks it,
 - ss via one square-TT + fp16 reduce per chunk on DVE; rn = exp(-0.5*ln(ss))
   on ACT in fp16; normalize+transpose fused: fnT = nat.T @ diag(rn) (diag
   built by one wide GPSIMD affine_select per chunk), fnT stored bf16 in 4
   groups of [128,2048] (one PSUM tile + wide casts per group),
 - loop is q-OUTER: per q-pass the fnT group is built once and 8 M-tiles
   stream matmul->exp back-to-back; group builds/ss/rn are emitted at
   scheduled (q,m) points so in-order engine queues never stall the exps;
   the q=2/q=3 passes are interleaved per-m so the DVE fold+tree work
   spreads out instead of piling into a tail,
 - bf16 mains ([128,512] into PSUM fp32); fp16 nat keeps the S error ~3e-4
   (bf16-nat would be 1.2e-3, too close to the 0.0031 min margin); self
   column pre-masked by an accumulating (-BIG*I) matmul so exp gives 0,
 - ONE ACT pass per psum tile: E = exp(w*S) in fp16 with fused row-sum accum.
   Z_i = sum_j E_ij; loss_i = ln Z_i - ln E_pos_i (E_pos = E at the positive,
   extracted by a fused mult-by-I + row-sum scalar_tensor_tensor),
 - prec1: row-max of E (fp16 TT-max folds + tree) vs E_pos*1.012: exact for
   any data whose min wrong-margin exceeds the matmul error (verified
   0.0031 in S units vs ~6e-4 error; threshold margin ~2x on both sides),
 - ACT activation tables pinned to the single set that holds {exp, ln, copy}
   so there is exactly one ACT_TABLE_LOAD.
 (note: tensor_tensor_reduce with op1=max crashes the device - do not use.)
"""
import numpy as np
from contextlib import ExitStack

import concourse.bass as bass
import concourse.tile as tile
from concourse import bacc, mybir
from concourse import hw_specs
from concourse.bass_utils import run_bass_kernel_spmd

F32 = mybir.dt.float32
F16 = mybir.dt.float16
BF16 = mybir.dt.bfloat16
AF = mybir.ActivationFunctionType
ALU = mybir.AluOpType

N_CORES = 8
B, C, D = 4096, 2, 128
N = B * C
ROWS = N // N_CORES
MT = ROWS // 128               # 8 M-tiles per core
JT = N // 512                  # 16 column tiles of 512
QT = 4                         # psum rounds per M-tile ([128,2048] each)
POS_OFF = N // 2
NEG_BIG = 60000.0              # fits fp16; exp(w*(S-NEG_BIG)) == 0
CORR_THR = 1.012

# --- tuning knobs ---
NCHUNK = 8                     # phase-1 chunks (8 nat tiles of 128 rows each)
TREE_STOP = 512                # TT-max tree -> tensor_reduce switch width

_cache = {}
_act_tables_patched = False


def _pin_act_tables():
    """Force every activation in this process onto the one table set that
    contains exp+ln+copy, so bacc emits a single ACT_TABLE_LOAD."""
    global _act_tables_patched
    if _act_tables_patched:
        return
    orig = hw_specs.get_activation_tables
    keep = "natural_log_exp_and_others"
    pin = {AF.Exp, AF.Ln, AF.Square, AF.Copy, AF.Identity}

    def patched(arch):
        tabs = orig(arch)
        if keep not in tabs:
            return tabs
        return {name: (funcs if name == keep else funcs - pin)
                for name, funcs in tabs.items()}

    hw_specs.get_activation_tables = patched
    bacc.get_activation_tables = patched
    _act_tables_patched = True


def _build_program(w: float, b: float):
    _pin_act_tables()
    nc = bacc.Bacc("TRN2", target_bir_lowering=False, debug=False,
                   enable_asserts=True, num_devices=N_CORES)

    # chunk-major, partition-contiguous layout: 2KB DMA lines per partition
    d_feat = nc.dram_tensor("feat", [NCHUNK, 128, (64 // NCHUNK) * D], F16,
                            kind="ExternalInput").ap()
    d_identf = nc.dram_tensor("identf", [128, 128], F16, kind="ExternalInput").ap()
    d_identb = nc.dram_tensor("identb", [128, 128], BF16, kind="ExternalInput").ap()
    d_negbig = nc.dram_tensor("negbig", [128, 128], BF16, kind="ExternalInput").ap()
    o_loss = nc.dram_tensor("loss_out", [128, MT], F32, kind="ExternalOutput").ap()
    o_corr = nc.dram_tensor("corr_out", [128, MT], F32, kind="ExternalOutput").ap()

    TPC = 64 // NCHUNK

    with tile.TileContext(nc) as tc, ExitStack() as ctx:
        consts = ctx.enter_context(tc.tile_pool(name="consts", bufs=1))
        natp = ctx.enter_context(tc.tile_pool(name="nat", bufs=1))
        fntp = ctx.enter_context(tc.tile_pool(name="fnt", bufs=1))
        stats = ctx.enter_context(tc.tile_pool(name="stats", bufs=1))
        scrp = ctx.enter_context(tc.tile_pool(name="scr", bufs=2))
        diagp = ctx.enter_context(tc.tile_pool(name="diag", bufs=8))
        ep = ctx.enter_context(tc.tile_pool(name="ep", bufs=14))
        treep = ctx.enter_context(tc.tile_pool(name="tree", bufs=8))
        treep2 = ctx.enter_context(tc.tile_pool(name="tree2", bufs=8))
        trp2 = ctx.enter_context(tc.tile_pool(name="tr2", bufs=2))
        psum = ctx.enter_context(tc.tile_pool(name="psum", bufs=2, space="PSUM"))

        identf = consts.tile([128, 128], F16, tag="identf")
        identb = consts.tile([128, 128], BF16, tag="identb")
        negbig = consts.tile([128, 128], BF16, tag="negbig")

        lnss = stats.tile([128, 64], F32, tag="lnss")
        rn16 = stats.tile([128, 64], F16, tag="rn16")
        zacc = stats.tile([128, MT * QT], F32, tag="zacc")
        epos = stats.tile([128, MT], F32, tag="epos")
        emax = stats.tile([128, MT], F32, tag="emax")

        # ---------- phase 1: load, sumsq, rnorm ----------
        # Chunk DMAs spread across engines (each engine issues on its own HW
        # DGE queue -> parallel transfers). Per chunk, sumsq = one square TT +
        # one fp16 reduce on DVE; rn on ACT. ss/rn are emitted at scheduled
        # points so the in-order ACT/DVE queues never block the exp stream.
        nat = [None] * NCHUNK
        ss16 = stats.tile([128, 64], F16, tag="ss16")
        dma_eng = [nc.sync, nc.scalar, nc.gpsimd]
        for cch in range(NCHUNK):
            nchunk = natp.tile([128, TPC, 128], F16, tag=f"nat{cch}")
            if cch < 4:
                # early chunks gate the ramp: one part per DMA-capable engine
                # queue so the three transfers run in parallel
                for k, (t0, t1) in enumerate(((0, 3), (3, 6), (6, 8))):
                    dma_eng[(cch + k) % 3].dma_start(
                        out=nchunk[:, t0:t1, :],
                        in_=d_feat[cch, :, 128 * t0:128 * t1])
            else:
                dma_eng[cch % 3].dma_start(out=nchunk[:], in_=d_feat[cch])
            nat[cch] = nchunk
        # consts are needed later than the feature chunks; issue them after
        nc.sync.dma_start(out=identf[:], in_=d_identf)
        nc.scalar.dma_start(out=identb[:], in_=d_identb)
        nc.gpsimd.dma_start(out=negbig[:], in_=d_negbig)

        # PE warm-up: dummy matmuls on zeroed tiles while the feature DMAs are
        # in flight, so the HAM activity window upclocks the PE (1.2->2.4 GHz)
        # before the first real transpose arrives.
        jw = consts.tile([128, 128], BF16, tag="jw")
        jr = consts.tile([128, 512], BF16, tag="jr")
        nc.vector.memset(jw[:], 0.0)
        nc.vector.memset(jr[:], 0.0)
        pjunk = psum.tile([128, 2048], F32, tag="psum")
        for _ in range(32):
            nc.tensor.matmul(pjunk[:, 0:512], jw[:], jr[:],
                             start=True, stop=True)

        def ss_chunk(cch):
            sl = slice(cch * TPC, (cch + 1) * TPC)
            sq = scrp.tile([128, TPC, 128], F16, tag="sq1024")
            nc.vector.tensor_tensor(out=sq[:], in0=nat[cch][:],
                                    in1=nat[cch][:], op=ALU.mult)
            with nc.allow_low_precision(reason="sumsq of 128 fp16 squares; "
                                        "rel err ~1e-3 is fine for rnorm"):
                nc.vector.tensor_reduce(out=ss16[:, sl], in_=sq[:],
                                        axis=mybir.AxisListType.X, op=ALU.add)

        def rn_chunks(lo, hi):
            sl = slice(lo * TPC, hi * TPC)
            # rn = ss^-1/2 = exp(-0.5*ln(ss)), output directly in fp16
            nc.scalar.activation(out=lnss[:, sl], in_=ss16[:, sl], func=AF.Ln)
            nc.scalar.activation(out=rn16[:, sl], in_=lnss[:, sl],
                                 func=AF.Exp, bias=0.0, scale=-0.5)

        def rn_chunk(cch):
            rn_chunks(cch, cch + 1)

        for cch in (0, 1):
            ss_chunk(cch)
            rn_chunk(cch)

        # ---------- transpose (+normalize) to fnT bf16, in groups of 2048 ----------
        # One [128,2048] PSUM tile holds 16 sub-transposes (4 j-tiles); one wide
        # GPSIMD affine_select per chunk builds all 8 diag(rn) tiles at once;
        # two [128,1024] casts convert psum fp32 -> sbuf bf16.
        fntg = {}

        def ensure_group(gq):
            if gq in fntg:
                return fntg[gq]
            pt = psum.tile([128, 2048], F32, tag="psum")
            for half in range(2):
                cch = 2 * gq + half
                dt8 = diagp.tile([128, TPC, 128], F16, tag="dt8")
                nc.gpsimd.affine_select(
                    out=dt8[:],
                    in_=rn16[:, cch * TPC:(cch + 1) * TPC].to_broadcast(
                        (128, TPC, 128)),
                    compare_op=ALU.is_equal, fill=0.0, base=0,
                    pattern=[[0, TPC], [-1, 128]], channel_multiplier=1)
                for t in range(TPC):
                    k = half * TPC + t
                    nc.tensor.matmul(pt[:, k * 128:(k + 1) * 128],
                                     nat[cch][:, t, :], dt8[:, t, :],
                                     start=True, stop=True)
            gtile = fntp.tile([128, 2048], BF16, tag=f"fntg{gq}")
            if gq == 0:
                # head: DVE is busy with sumsq; ACT is idle until the first exp
                nc.scalar.copy(gtile[:, 0:1024], pt[:, 0:1024])
                nc.scalar.copy(gtile[:, 1024:2048], pt[:, 1024:2048])
            else:
                nc.vector.tensor_copy(gtile[:, 0:1024], pt[:, 0:1024])
                nc.vector.tensor_copy(gtile[:, 1024:2048], pt[:, 1024:2048])
            fntg[gq] = gtile
            return gtile

        # ---------- phase 2 (q outer, m inner): S block, exp+sum, E_pos, max ----
        # Per q-pass the fnT group is built once, then 8 M-tiles stream matmul->
        # exp back-to-back; the next group's transposes overlap the exp stream.
        eblk = [[None] * QT for _ in range(MT)]
        rmax = [None] * MT
        # emission schedule: (q, m) -> actions run AFTER that m's exp is
        # emitted. Groups/ss/rn are spread so each engine's in-order queue
        # digests them during exp slack instead of blocking the next exp.
        post_exp = {
            (0, 0): [lambda: ss_chunk(2), lambda: ss_chunk(3)],
            (0, 1): [lambda: rn_chunks(2, 4)],
            (0, 2): [lambda: ss_chunk(4), lambda: ss_chunk(5)],
            (0, 3): [lambda: ensure_group(1)],
            (0, 4): [lambda: rn_chunks(4, 6)],
            (0, 5): [lambda: ss_chunk(6), lambda: ss_chunk(7)],
            (0, 7): [lambda: ensure_group(2), lambda: rn_chunks(6, 8)],
            (1, 2): [lambda: ensure_group(3)],
        }

        # passes 1-3 run on a software-pipelined diagonal so the per-m DVE
        # fold+tree work spreads across the exp stream instead of piling up
        # after the last exp
        sched = [(0, m) for m in range(MT)]
        sched += [(1, 0),
                  (1, 1), (2, 0),
                  (1, 2), (2, 1),
                  (1, 3), (2, 2),
                  (1, 4), (2, 3), (3, 0),
                  (1, 5), (2, 4), (3, 1),
                  (1, 6), (2, 5), (3, 2),
                  (1, 7), (2, 6), (3, 3),
                  (2, 7), (3, 4),
                  (3, 5), (3, 6), (3, 7)]
        for q, m in sched:
            grp = ensure_group(q)
            if True:
                lhsT = fntg[0][:, 128 * m:128 * (m + 1)]
                pm = psum.tile([128, 2048], F32, tag="psum")
                # tiny keep-warm matmul: lands in psum the real mains then
                # overwrite (start=True); keeps the HAM activity window from
                # seeing an idle PE and re-throttling the clock
                nc.tensor.matmul(pm[0:64, 0:512], jw[:, 0:64], jr[:],
                                 start=True, stop=True, skip_group_check=True)
                for jj in range(4):
                    nc.tensor.matmul(pm[:, jj * 512:(jj + 1) * 512], lhsT,
                                     grp[:, jj * 512:(jj + 1) * 512],
                                     start=True, stop=True)
                if q == 0:
                    # self column block: accumulate -BIG*I
                    nc.tensor.matmul(pm[:, 128 * m:128 * (m + 1)], identb[:],
                                     negbig[:], start=False, stop=True,
                                     skip_group_check=True)
                et = ep.tile([128, 2048], F16, tag="E")
                nc.scalar.activation(out=et[:], in_=pm[:], func=AF.Exp, scale=w,
                                     accum_out=zacc[:, QT * m + q:QT * m + q + 1])
                eblk[m][q] = et
                if q == 2:
                    # E at the positive column (col 4096+128m -> offset 128m in q=2)
                    escr = scrp.tile([128, 128], F16, tag="escr")
                    nc.vector.scalar_tensor_tensor(
                        out=escr[:], in0=et[:, 128 * m:128 * (m + 1)],
                        scalar=1.0, in1=identf[:], op0=ALU.mult, op1=ALU.mult,
                        accum_out=epos[:, m:m + 1])
                # running row-max, work balanced across the (q,m) slots so no
                # slot exceeds the ~2us exp cadence: (1,m) one 2048 fold;
                # (2,m) fold + shrink to 1024; (3,m) fold q3 in halves + tree
                if q == 1:
                    rm = treep.tile([128, 2048], F16, tag="rmax")
                    nc.vector.tensor_tensor(out=rm[:], in0=eblk[m][0][:],
                                            in1=et[:], op=ALU.max)
                    rmax[m] = rm
                    eblk[m][0] = eblk[m][1] = None
                elif q == 2:
                    nc.vector.tensor_tensor(out=rmax[m][:], in0=rmax[m][:],
                                            in1=et[:], op=ALU.max)
                    rm1k = treep2.tile([128, 1024], F16, tag="rmax1k")
                    nc.vector.tensor_tensor(out=rm1k[:], in0=rmax[m][:, 0:1024],
                                            in1=rmax[m][:, 1024:2048],
                                            op=ALU.max)
                    rmax[m] = rm1k
                    eblk[m][q] = None
                elif q == 3:
                    e3h = trp2.tile([128, 1024], F16, tag="e3h")
                    nc.vector.tensor_tensor(out=e3h[:], in0=et[:, 0:1024],
                                            in1=et[:, 1024:2048], op=ALU.max)
                    t1 = trp2.tile([128, 512], F16, tag="t1")
                    nc.vector.scalar_tensor_tensor(
                        out=t1[:], in0=e3h[:, 0:512], scalar=1.0,
                        in1=e3h[:, 512:1024], op0=ALU.mult, op1=ALU.max)
                    t2 = trp2.tile([128, 512], F16, tag="t2")
                    nc.vector.tensor_tensor(out=t2[:], in0=rmax[m][:, 0:512],
                                            in1=rmax[m][:, 512:1024],
                                            op=ALU.max)
                    t3 = trp2.tile([128, 512], F16, tag="t3")
                    nc.vector.tensor_tensor(out=t3[:], in0=t1[:], in1=t2[:],
                                            op=ALU.max)
                    nc.vector.tensor_reduce(out=emax[:, m:m + 1], in_=t3[:],
                                            axis=mybir.AxisListType.X,
                                            op=ALU.max)
                    eblk[m][q] = None
                for act in post_exp.get((q, m), []):
                    act()

        # ---------- finals ----------
        z = stats.tile([128, MT], F32, tag="z")
        nc.vector.tensor_reduce(out=z[:], in_=zacc[:].rearrange("p (m q) -> p m q", q=QT),
                                axis=mybir.AxisListType.X, op=ALU.add)
        lnz = stats.tile([128, MT], F32, tag="lnz")
        nc.scalar.activation(out=lnz[:], in_=z[:], func=AF.Ln)
        lnpos = stats.tile([128, MT], F32, tag="lnpos")
        nc.scalar.activation(out=lnpos[:], in_=epos[:], func=AF.Ln)
        lossb = stats.tile([128, MT], F32, tag="lossb")
        nc.vector.tensor_tensor(out=lossb[:], in0=lnz[:], in1=lnpos[:],
                                op=ALU.subtract)
        # corr = (E_pos * CORR_THR) >= rowmax(E)  (pos column itself is in the max)
        corrb = stats.tile([128, MT], F32, tag="corrb")
        nc.vector.scalar_tensor_tensor(out=corrb[:], in0=epos[:], scalar=CORR_THR,
                                       in1=emax[:], op0=ALU.mult, op1=ALU.is_ge)
        nc.sync.dma_start(out=o_loss, in_=lossb[:])
        nc.sync.dma_start(out=o_corr, in_=corrb[:])

    nc.compile()
    return nc


def _get_program(w: float, b: float):
    key = (w, b)
    if key not in _cache:
        _cache[key] = _build_program(w, b)
    return _cache[key]


def make_in_maps(features: np.ndarray):
    import ml_dtypes
    feat = np.ascontiguousarray(
        np.swapaxes(np.asarray(features, np.float32), 0, 1).reshape(N, D)
    ).astype(np.float16)
    identf = np.eye(128, dtype=np.float16)
    identb = np.eye(128, dtype=ml_dtypes.bfloat16)
    negbig = (-NEG_BIG * np.eye(128)).astype(ml_dtypes.bfloat16)
    TPC = 64 // NCHUNK
    in_maps = []
    for c in range(N_CORES):
        rot = np.roll(feat, -ROWS * c, axis=0) if c else feat
        # chunk-major, partition-contiguous: [c, p, t*D+d] <- rot[(c*TPC+t)*128+p, d]
        fdma = np.ascontiguousarray(
            rot.reshape(NCHUNK, TPC, 128, D).transpose(0, 2, 1, 3)
               .reshape(NCHUNK, 128, TPC * D))
        in_maps.append({"feat": fdma, "identf": identf,
                        "identb": identb, "negbig": negbig})
    return in_maps


def kernel(features: np.ndarray, w: np.ndarray, b: np.ndarray):
    features = np.asarray(features, dtype=np.float32)
    wf = float(np.asarray(w)); bf = float(np.asarray(b))
    assert features.shape == (B, C, D), features.shape

    nc = _get_program(wf, bf)
    in_maps = make_in_maps(features)
    res = run_bass_kernel_spmd(nc, in_maps, list(range(N_CORES)))

    loss_sum = 0.0
    corr_sum = 0.0
    for c in range(N_CORES):
        loss_sum += float(res.results[c]["loss_out"].astype(np.float64).sum())
        corr_sum += float(res.results[c]["corr_out"].astype(np.float64).sum())
    return (np.float32(loss_sum / N), np.float32(100.0 * corr_sum / N))


if __name__ == "__main__":
    import jax
    key = jax.random.key(0)
    k1, = jax.random.split(key, 1)
    feats = np.asarray(jax.random.normal(k1, (B, C, D), dtype=np.float32))
    out = kernel(features=feats, w=np.float32(10.0), b=np.float32(-5.0))
    print("loss, prec1 =", out)
